# revision 3
# baseline (speedup 1.0000x reference)
import sys

sys.path.insert(0, "/opt/trn_rl_repo")
import numpy as np
from contextlib import ExitStack

import concourse.bass as bass
import concourse.bacc as bacc
import concourse.tile as tile
from concourse import mybir
from concourse.bass_utils import run_bass_kernel_spmd

fp32 = mybir.dt.float32
fp32r = mybir.dt.float32r
ts = bass.ts
Exp = mybir.ActivationFunctionType.Exp
ADD = mybir.AluOpType.add
MULT = mybir.AluOpType.mult

# PerceiverAttention, GQA. Hardcoded problem shapes.
H = 2048            # hidden
NH = 32             # query heads
D = 64              # head dim
G = 8               # kv groups (rep=4, repeat_interleave: head h -> group h//4)
DKV = G * D         # 512
NLAT = 64
SEQ = 4096
KV = SEQ + NLAT     # 4160 = [context; latents]
HC = H // 128       # 16 h-chunks
SUP = 512           # kv rows per streamed super-block
NSUP = SEQ // SUP   # 8
ESC = 0.125         # 1/sqrt(head_dim)

_cache = {}


def _build():
    nc = bacc.Bacc("TRN2", target_bir_lowering=False, debug=False, num_devices=1)
    lat_d = nc.dram_tensor("lat", [NLAT, H], fp32, kind="ExternalInput").ap()
    ctx_d = nc.dram_tensor("ctx", [SEQ, H], fp32, kind="ExternalInput").ap()
    wq_d = nc.dram_tensor("wq", [H, H], fp32, kind="ExternalInput").ap()
    wk_d = nc.dram_tensor("wk", [H, DKV], fp32, kind="ExternalInput").ap()
    wv_d = nc.dram_tensor("wv", [H, DKV], fp32, kind="ExternalInput").ap()
    wo_d = nc.dram_tensor("wo", [H, H], fp32, kind="ExternalInput").ap()
    id_d = nc.dram_tensor("ident", [128, 128], fp32, kind="ExternalInput").ap()
    out_d = nc.dram_tensor("out", [NLAT, H], fp32, kind="ExternalOutput").ap()

    with tile.TileContext(nc) as tc, ExitStack() as ctx:
        sb1 = ctx.enter_context(tc.tile_pool(name="sb1", bufs=1))
        sb2 = ctx.enter_context(tc.tile_pool(name="sb2", bufs=2))
        ps = ctx.enter_context(tc.tile_pool(name="ps", bufs=2, space="PSUM"))

        # persistent SBUF
        wk_sb = sb1.tile([128, HC, DKV], fp32r)       # [h-part, hc, d_kv]
        wv_sb = sb1.tile([128, HC, DKV], fp32r)
        latT = sb1.tile([128, HC, NLAT], fp32r)       # lat^T
        qT = sb1.tile([128, 4, 256], fp32r)           # chunk c: groups 2c @p0, 2c+1 @p64; free=(hi,lat)
        o_acc = sb1.tile([128, G, 256], fp32)         # rows 0:64 attn-out^T, row 64 denom
        ctx_f = sb1.tile([128, 4, H], fp32)           # current super, natural layout
        ctxT = sb1.tile([128, HC, SUP], fp32r)        # ctx^T current super
        kT = sb1.tile([128, 4, SUP], fp32r)           # K^T current super (chunk c: groups 2c/2c+1)
        v1 = sb1.tile([128, 4, G, D + 1], fp32r)      # V' per kv-subchunk/group, col 64 = ones
        kTt = sb1.tile([128, 4, NLAT], fp32r)         # K_tail^T (latent kv rows)
        v1t = sb1.tile([NLAT, G, D + 1], fp32r)
        id32 = sb1.tile([128, 128], fp32)
        den = sb1.tile([1, 256], fp32)
        ones1 = sb1.tile([1, NLAT], fp32)

        def t64(tag="q64"):
            return sb1.tile([NLAT, H], fp32, tag=tag, name=tag)

        def tattn():
            return sb1.tile([128, HC, NLAT], fp32r, tag="latT2", name="latT2")

        def wst():
            return sb2.tile([128, 512], fp32, tag="wst", name="wst")

        def wr():
            return sb2.tile([128, 512], fp32r, tag="wr", name="wr")

        def et():
            return sb2.tile([128, 256], fp32r, tag="et", name="et")

        def p(tag):
            return ps.tile([128, 512], fp32, tag=tag, name=tag)

        nc.sync.dma_start(out=id32, in_=id_d)
        lat_sb = t64("lat64")
        nc.sync.dma_start(out=lat_sb, in_=lat_d)
        nc.vector.memset(o_acc, 0.0)
        nc.vector.memset(ones1, 1.0)
        onesc = sb1.tile([128, 1], fp32)
        nc.vector.memset(onesc, 1.0)
        for kc in range(4):
            for g in range(G):
                nc.scalar.copy(out=v1[:, kc, g, 64:65], in_=onesc)
        for g in range(G):
            nc.scalar.copy(out=v1t[:, g, 64:65], in_=onesc[0:NLAT, :])

        # Wk/Wv resident as fp32r
        for wd, wsb in ((wk_d, wk_sb), (wv_d, wv_sb)):
            for hc in range(HC):
                w = wst()
                nc.sync.dma_start(out=w, in_=wd[ts(hc, 128), :])
                nc.vector.tensor_copy(out=wsb[:, hc, :], in_=w)

        # lat^T via PE transpose (fp32 in, round to fp32r at drain)
        for hc in range(HC):
            tp = p("tr")
            nc.tensor.transpose(tp[:, 0:NLAT], lat_sb[0:NLAT, ts(hc, 128)],
                                id32[0:NLAT, 0:NLAT])
            nc.scalar.copy(out=latT[:, hc, :], in_=tp[:, 0:NLAT])

        # Q = lat @ Wq, streaming Wq chunks
        q_sb = t64("q64")
        for qo in range(4):
            qp = p("kv")
            for hc in range(HC):
                w = wst()
                nc.sync.dma_start(out=w, in_=wq_d[ts(hc, 128), ts(qo, 512)])
                wq_r = wr()
                nc.vector.tensor_copy(out=wq_r, in_=w)
                nc.tensor.matmul(qp[0:NLAT, :], latT[:, hc, :], wq_r,
                                 start=(hc == 0), stop=(hc == HC - 1))
            nc.scalar.copy(out=q_sb[0:NLAT, ts(qo, 512)], in_=qp[0:NLAT, :])

        # Q^T per head -> grouped layout
        for h in range(NH):
            g, hi = h // 4, h % 4
            c, b0 = g // 2, 64 * (g % 2)
            tp = p("tr")
            nc.tensor.transpose(tp[0:NLAT, 0:NLAT], q_sb[0:NLAT, ts(h, D)],
                                id32[0:NLAT, 0:NLAT])
            nc.scalar.copy(out=qT[b0:b0 + 64, c, ts(hi, 64)],
                           in_=tp[0:NLAT, 0:NLAT])

        # K_tail/V_tail from latents (kv rows 4096:4160)
        ktp = p("kv")
        for hc in range(HC):
            nc.tensor.matmul(ktp[0:NLAT, :], latT[:, hc, :], wk_sb[:, hc, :],
                             start=(hc == 0), stop=(hc == HC - 1))
        kt_f = t64("q64")
        nc.scalar.copy(out=kt_f[0:NLAT, 0:DKV], in_=ktp[0:NLAT, :])
        for dc in range(4):
            tp = p("tr")
            nc.tensor.transpose(tp[:, 0:NLAT], kt_f[0:NLAT, ts(dc, 128)],
                                id32[0:NLAT, 0:NLAT])
            nc.scalar.copy(out=kTt[:, dc, :], in_=tp[:, 0:NLAT])
        vtp = p("kv")
        for hc in range(HC):
            nc.tensor.matmul(vtp[0:NLAT, :], latT[:, hc, :], wv_sb[:, hc, :],
                             start=(hc == 0), stop=(hc == HC - 1))
        for g in range(G):
            nc.scalar.copy(out=v1t[:, g, 0:D], in_=vtp[0:NLAT, ts(g, D)])

        def attend(g, j, lhsT_k, rhs_q, v_ap, kv_rows, first, last):
            sp = p("s")
            nc.tensor.matmul(sp[0:kv_rows, 0:256], lhsT_k, rhs_q,
                             start=True, stop=True)
            e = et()
            nc.scalar.activation(out=e[0:kv_rows, :], in_=sp[0:kv_rows, 0:256],
                                 func=Exp, scale=ESC)
            op = p("o") if first else attend.op
            attend.op = op
            nc.tensor.matmul(op[0:D + 1, 0:256], v_ap, e[0:kv_rows, :],
                             start=first, stop=last)
            if last:
                nc.vector.tensor_tensor(out=o_acc[0:D + 1, g, :],
                                        in0=o_acc[0:D + 1, g, :],
                                        in1=op[0:D + 1, 0:256], op=ADD)

        # streamed kv supers over context
        for s in range(NSUP):
            for kc in range(4):
                nc.sync.dma_start(out=ctx_f[:, kc, :],
                                  in_=ctx_d[ts(4 * s + kc, 128), :])
            for kc in range(4):
                for hc in range(HC):
                    tp = p("tr")
                    nc.tensor.transpose(tp[:, 0:128], ctx_f[:, kc, ts(hc, 128)],
                                        id32)
                    nc.scalar.copy(out=ctxT[:, hc, ts(kc, 128)], in_=tp[:, 0:128])
            for dc in range(4):
                kp = p("kv")
                for hc in range(HC):
                    nc.tensor.matmul(kp, wk_sb[:, hc, ts(dc, 128)],
                                     ctxT[:, hc, :],
                                     start=(hc == 0), stop=(hc == HC - 1))
                nc.scalar.copy(out=kT[:, dc, :], in_=kp)
            for kc in range(4):
                vp = p("kv")
                for hc in range(HC):
                    nc.tensor.matmul(vp, ctxT[:, hc, ts(kc, 128)],
                                     wv_sb[:, hc, :],
                                     start=(hc == 0), stop=(hc == HC - 1))
                for g in range(G):
                    nc.scalar.copy(out=v1[:, kc, g, 0:D], in_=vp[:, ts(g, D)])
            for g in range(G):
                c, b0 = g // 2, 64 * (g % 2)
                for j in range(4):
                    attend(g, j,
                           kT[b0:b0 + 64, c, ts(j, 128)],
                           qT[b0:b0 + 64, c, :],
                           v1[:, j, g, :], 128, j == 0, j == 3)

        # latent kv tail
        for g in range(G):
            c, b0 = g // 2, 64 * (g % 2)
            attend(g, 0,
                   kTt[b0:b0 + 64, c, :],
                   qT[b0:b0 + 64, c, :],
                   v1t[:, g, :], NLAT, True, True)

        # normalize: attn = exp / denom (row 64 of o_acc)
        for g in range(G):
            nc.scalar.copy(out=den, in_=o_acc[64:65, g, :])
            nc.vector.reciprocal(out=den, in_=den)
            bp = p("s")
            nc.tensor.matmul(bp[0:NLAT, 0:256], ones1, den, start=True, stop=True)
            nc.vector.tensor_tensor(out=o_acc[0:D, g, :], in0=o_acc[0:D, g, :],
                                    in1=bp[0:NLAT, 0:256], op=MULT)

        # assemble attn^T [h*64+d, lat] chunks
        att = tattn()
        for c in range(HC):
            h0, h1 = 2 * c, 2 * c + 1
            nc.scalar.copy(out=att[0:64, c, :],
                           in_=o_acc[0:D, h0 // 4, ts(h0 % 4, 64)])
            nc.scalar.copy(out=att[64:128, c, :],
                           in_=o_acc[0:D, h1 // 4, ts(h1 % 4, 64)])

        # out = attn_flat @ Wo, streaming Wo
        out_sb = t64("lat64")
        for qo in range(4):
            op = p("kv")
            for hc in range(HC):
                w = wst()
                nc.sync.dma_start(out=w, in_=wo_d[ts(hc, 128), ts(qo, 512)])
                wo_r = wr()
                nc.vector.tensor_copy(out=wo_r, in_=w)
                nc.tensor.matmul(op[0:NLAT, :], att[:, hc, :], wo_r,
                                 start=(hc == 0), stop=(hc == HC - 1))
            nc.scalar.copy(out=out_sb[0:NLAT, ts(qo, 512)], in_=op[0:NLAT, :])
            nc.sync.dma_start(out=out_d[:, ts(qo, 512)],
                              in_=out_sb[0:NLAT, ts(qo, 512)])

    nc.compile()
    return nc


def kernel(latents, context, Wq, Wk, Wv, Wo):
    if "nc" not in _cache:
        _cache["nc"] = _build()
    nc = _cache["nc"]
    ident = np.eye(128, dtype=np.float32)
    in_maps = [
        {
            "lat": np.ascontiguousarray(latents[i]),
            "ctx": np.ascontiguousarray(context[i]),
            "wq": Wq, "wk": Wk, "wv": Wv, "wo": Wo, "ident": ident,
        }
        for i in range(8)
    ]
    res = run_bass_kernel_spmd(nc, in_maps, list(range(8)))
    return np.stack([res.results[i]["out"] for i in range(8)], axis=0)


# revision 8
# speedup vs baseline: 1.0821x; 1.0821x over previous
import sys

sys.path.insert(0, "/opt/trn_rl_repo")
import numpy as np
from contextlib import ExitStack

import concourse.bass as bass
import concourse.bacc as bacc
import concourse.tile as tile
from concourse import mybir
from concourse.bass_utils import run_bass_kernel_spmd

fp32 = mybir.dt.float32
fp32r = mybir.dt.float32r
ts = bass.ts
Exp = mybir.ActivationFunctionType.Exp
ADD = mybir.AluOpType.add
MULT = mybir.AluOpType.mult

# PerceiverAttention, GQA. Hardcoded problem shapes.
H = 2048            # hidden
NH = 32             # query heads
D = 64              # head dim
G = 8               # kv groups (rep=4, repeat_interleave: head h -> group h//4)
DKV = G * D         # 512
NLAT = 64
SEQ = 4096
KV = SEQ + NLAT     # 4160 = [context; latents]
HC = H // 128       # 16 h-chunks
SUP = 512           # kv rows per streamed super-block
NSUP = SEQ // SUP   # 8
ESC = 0.125         # 1/sqrt(head_dim)

USE_GPSIMD_WQ = True

_cache = {}


def _build():
    nc = bacc.Bacc("TRN2", target_bir_lowering=False, debug=False, num_devices=1)
    lat_d = nc.dram_tensor("lat", [NLAT, H], fp32, kind="ExternalInput").ap()
    ctx_d = nc.dram_tensor("ctx", [SEQ, H], fp32, kind="ExternalInput").ap()
    wq_d = nc.dram_tensor("wq", [H, H], fp32, kind="ExternalInput").ap()
    wk_d = nc.dram_tensor("wk", [H, DKV], fp32, kind="ExternalInput").ap()
    wv_d = nc.dram_tensor("wv", [H, DKV], fp32, kind="ExternalInput").ap()
    wo_d = nc.dram_tensor("wo", [H, H], fp32, kind="ExternalInput").ap()
    id_d = nc.dram_tensor("ident", [128, 128], fp32, kind="ExternalInput").ap()
    out_d = nc.dram_tensor("out", [NLAT, H], fp32, kind="ExternalOutput").ap()

    wq_eng_attr = "gpsimd" if USE_GPSIMD_WQ else "scalar"

    with tile.TileContext(nc) as tc, ExitStack() as ctx:
        sb1 = ctx.enter_context(tc.tile_pool(name="sb1", bufs=1))
        sb2 = ctx.enter_context(tc.tile_pool(name="sb2", bufs=2))
        ps = ctx.enter_context(tc.tile_pool(name="ps", bufs=2, space="PSUM"))

        # persistent SBUF
        wk_sb = sb1.tile([128, HC, DKV], fp32r)       # [h-part, hc, d_kv]
        wv_sb = sb1.tile([128, HC, DKV], fp32r)
        latT = sb1.tile([128, HC, NLAT], fp32r)       # lat^T
        qT = sb1.tile([128, 4, 256], fp32r)           # chunk c: groups 2c @p0, 2c+1 @p64
        o_acc = sb1.tile([128, G, 256], fp32)         # rows 0:64 attn-out^T, row 64 denom
        ctxT = sb1.tile([128, HC, SUP], fp32r)        # ctx^T current super
        kT = sb1.tile([128, 4, SUP], fp32r)           # K^T current super
        v1 = sb1.tile([128, 4, G, D + 1], fp32r)      # V' per kv-subchunk/group, col 64 ones
        kTt = sb1.tile([128, 4, NLAT], fp32r)         # K_tail^T (latent kv rows)
        v1t = sb1.tile([NLAT, G, D + 1], fp32r)
        id32 = sb1.tile([128, 128], fp32)
        id_r = sb1.tile([128, 128], fp32r)
        att = sb1.tile([128, HC, NLAT], fp32)         # attn^T assembled (fp32 for Wo mm)
        den = sb1.tile([1, 1024], fp32)
        ones1 = sb1.tile([1, NLAT], fp32)
        onesc = sb1.tile([128, 1], fp32)

        def stg():
            return sb2.tile([128, H], fp32, tag="stg", name="stg")

        def ctr():
            return sb2.tile([128, H], fp32r, tag="ctr", name="ctr")

        def wst():
            return sb2.tile([128, 512], fp32, tag="wst", name="wst")

        def wr():
            return sb2.tile([128, 512], fp32r, tag="wr", name="wr")

        def qr():
            return sb2.tile([NLAT, 512], fp32r, tag="qr", name="qr")

        def et():
            return sb2.tile([128, 256], fp32r, tag="et", name="et")

        def outst():
            return sb2.tile([NLAT, 512], fp32, tag="outst", name="outst")

        def p(tag):
            return ps.tile([128, 512], fp32, tag=tag, name=tag)

        def ptr():
            return ps.tile([128, 4, 128], fp32r, tag="tr", name="tr")

        # ---- startup DMAs -------------------------------------------------
        # sync queue: ident, lat, then ctx supers (in super loop)
        # scalar queue: wk, wv, then wo halves (at tail)
        # gpsimd queue: wq
        nc.sync.dma_start(out=id32, in_=id_d)
        nc.vector.tensor_copy(out=id_r, in_=id32)
        lat_st = stg()
        nc.sync.dma_start(out=lat_st[0:NLAT, :], in_=lat_d)
        lat_r = ctr()
        nc.vector.tensor_copy(out=lat_r[0:NLAT, :], in_=lat_st[0:NLAT, :])

        nc.vector.memset(o_acc, 0.0)
        nc.vector.memset(ones1, 1.0)
        nc.vector.memset(onesc, 1.0)
        for kc in range(4):
            for g in range(G):
                nc.scalar.copy(out=v1[:, kc, g, 64:65], in_=onesc)
        for g in range(G):
            nc.scalar.copy(out=v1t[:, g, 64:65], in_=onesc[0:NLAT, :])

        # Wk/Wv resident as fp32r (scalar queue DMA, DVE cast)
        for wd, wsb in ((wk_d, wk_sb), (wv_d, wv_sb)):
            for hc in range(HC):
                w = wst()
                nc.scalar.dma_start(out=w, in_=wd[ts(hc, 128), :])
                nc.vector.tensor_copy(out=wsb[:, hc, :], in_=w)

        # lat^T: fp32r transposes, batched drains
        for hq in range(4):
            tp = ptr()
            for i in range(4):
                hc = 4 * hq + i
                nc.tensor.transpose(tp[:, i, 0:NLAT],
                                    lat_r[0:NLAT, ts(hc, 128)],
                                    id_r[0:NLAT, 0:NLAT])
            nc.vector.tensor_copy(out=latT[:, ts(hq, 4), :], in_=tp[:, :, 0:NLAT])

        def super_kv(s):
            # stream ctx super s: DMA (sync q) -> cast (DVE) -> transpose (PE)
            # -> ctxT (DVE drain); then K-proj / V-proj.
            for kc in range(4):
                st = stg()
                nc.sync.dma_start(out=st, in_=ctx_d[ts(4 * s + kc, 128), :])
                cr = ctr()
                nc.vector.tensor_copy(out=cr, in_=st)
                for hq in range(4):
                    tp = ptr()
                    for i in range(4):
                        hc = 4 * hq + i
                        nc.tensor.transpose(tp[:, i, :], cr[:, ts(hc, 128)], id_r)
                    nc.vector.tensor_copy(out=ctxT[:, ts(hq, 4), ts(kc, 128)],
                                          in_=tp)
            for dc in range(4):
                kp = p("kv")
                for hc in range(HC):
                    nc.tensor.matmul(kp, wk_sb[:, hc, ts(dc, 128)], ctxT[:, hc, :],
                                     start=(hc == 0), stop=(hc == HC - 1))
                nc.scalar.copy(out=kT[:, dc, :], in_=kp)
            for kc in range(4):
                vp = p("kv")
                for hc in range(HC):
                    nc.tensor.matmul(vp, ctxT[:, hc, ts(kc, 128)], wv_sb[:, hc, :],
                                     start=(hc == 0), stop=(hc == HC - 1))
                for g in range(G):
                    nc.scalar.copy(out=v1[:, kc, g, 0:D], in_=vp[:, ts(g, D)])

        def attend(g, j, lhsT_k, rhs_q, v_ap, kv_rows, first, last):
            sp = p("s")
            nc.tensor.matmul(sp[0:kv_rows, 0:256], lhsT_k, rhs_q,
                             start=True, stop=True)
            e = et()
            nc.scalar.activation(out=e[0:kv_rows, :], in_=sp[0:kv_rows, 0:256],
                                 func=Exp, scale=ESC)
            op = p("o") if first else attend.op
            attend.op = op
            nc.tensor.matmul(op[0:D + 1, 0:256], v_ap, e[0:kv_rows, :],
                             start=first, stop=last)
            if last:
                nc.vector.tensor_tensor(out=o_acc[0:D + 1, g, :],
                                        in0=o_acc[0:D + 1, g, :],
                                        in1=op[0:D + 1, 0:256], op=ADD)

        def attend_super(s):
            for g in range(G):
                c, b0 = g // 2, 64 * (g % 2)
                for j in range(4):
                    attend(g, j,
                           kT[b0:b0 + 64, c, ts(j, 128)],
                           qT[b0:b0 + 64, c, :],
                           v1[:, j, g, :], 128, j == 0, j == 3)

        def q_and_tails():
            # Q = lat @ Wq (wq streamed on gpsimd queue), then Q^T, then
            # latent-kv tail K^T / V'.
            wq_eng = getattr(nc, wq_eng_attr)
            for qo in range(4):
                qp = p("kv")
                for hc in range(HC):
                    w = wst()
                    wq_eng.dma_start(out=w, in_=wq_d[ts(hc, 128), ts(qo, 512)])
                    wq_r = wr()
                    nc.scalar.copy(out=wq_r, in_=w)
                    nc.tensor.matmul(qp[0:NLAT, :], latT[:, hc, :], wq_r,
                                     start=(hc == 0), stop=(hc == HC - 1))
                q_r = qr()
                nc.scalar.copy(out=q_r, in_=qp[0:NLAT, :])
                # heads 8*qo .. 8*qo+7 live in this 512-col chunk; chunk c==qo
                for half in range(2):
                    b0 = NLAT * half
                    tp = ptr()
                    for i in range(4):
                        nc.tensor.transpose(tp[0:NLAT, i, 0:NLAT],
                                            q_r[:, ts(4 * half + i, D)],
                                            id_r[0:NLAT, 0:NLAT])
                    for i in range(4):
                        nc.scalar.copy(out=qT[b0:b0 + NLAT, qo, ts(i, D)],
                                       in_=tp[0:NLAT, i, 0:NLAT])
            # K_tail
            ktp = p("kv")
            for hc in range(HC):
                nc.tensor.matmul(ktp[0:NLAT, :], latT[:, hc, :], wk_sb[:, hc, :],
                                 start=(hc == 0), stop=(hc == HC - 1))
            kt_f = qr()
            nc.scalar.copy(out=kt_f, in_=ktp[0:NLAT, :])
            tp = ptr()
            for dc in range(4):
                nc.tensor.transpose(tp[:, dc, 0:NLAT], kt_f[:, ts(dc, 128)],
                                    id_r[0:NLAT, 0:NLAT])
            nc.vector.tensor_copy(out=kTt, in_=tp[:, :, 0:NLAT])
            # V_tail
            vtp = p("kv")
            for hc in range(HC):
                nc.tensor.matmul(vtp[0:NLAT, :], latT[:, hc, :], wv_sb[:, hc, :],
                                 start=(hc == 0), stop=(hc == HC - 1))
            for g in range(G):
                nc.scalar.copy(out=v1t[:, g, 0:D], in_=vtp[0:NLAT, ts(g, D)])

        for s in range(NSUP):
            super_kv(s)
            if s == 0:
                q_and_tails()
            attend_super(s)

        # latent kv tail attend
        for g in range(G):
            c, b0 = g // 2, 64 * (g % 2)
            attend(g, 0, kTt[b0:b0 + 64, c, :], qT[b0:b0 + 64, c, :],
                   v1t[:, g, :], NLAT, True, True)

        # normalize: attn = exp / denom (row 64 of o_acc); batched reciprocal
        for half in range(2):
            for gg in range(4):
                nc.scalar.copy(out=den[:, ts(gg, 256)],
                               in_=o_acc[64:65, 4 * half + gg, :])
            nc.vector.reciprocal(out=den, in_=den)
            for gg in range(4):
                g = 4 * half + gg
                bp = p("s")
                nc.tensor.matmul(bp[0:NLAT, 0:256], ones1, den[:, ts(gg, 256)],
                                 start=True, stop=True)
                nc.vector.tensor_tensor(out=o_acc[0:D, g, :],
                                        in0=o_acc[0:D, g, :],
                                        in1=bp[0:NLAT, 0:256], op=MULT)

        # assemble attn^T [h*64+d, lat] chunks (fp32, for fp32 Wo matmul)
        for c in range(HC):
            h0, h1 = 2 * c, 2 * c + 1
            nc.scalar.copy(out=att[0:64, c, :],
                           in_=o_acc[0:D, h0 // 4, ts(h0 % 4, 64)])
            nc.scalar.copy(out=att[64:128, c, :],
                           in_=o_acc[0:D, h1 // 4, ts(h1 % 4, 64)])

        # out = attn_flat @ Wo; Wo streamed fp32 on BOTH hwdge queues
        for qo in range(4):
            op = p("kv")
            for hc in range(HC):
                w = wst()
                eng = nc.sync if (qo * HC + hc) % 2 == 0 else nc.scalar
                eng.dma_start(out=w, in_=wo_d[ts(hc, 128), ts(qo, 512)])
                nc.tensor.matmul(op[0:NLAT, :], att[:, hc, :], w,
                                 start=(hc == 0), stop=(hc == HC - 1))
            o_sb = outst()
            nc.scalar.copy(out=o_sb, in_=op[0:NLAT, :])
            nc.sync.dma_start(out=out_d[:, ts(qo, 512)], in_=o_sb)

    nc.compile()
    return nc


def kernel(latents, context, Wq, Wk, Wv, Wo):
    if "nc" not in _cache:
        _cache["nc"] = _build()
    nc = _cache["nc"]
    ident = np.eye(128, dtype=np.float32)
    in_maps = [
        {
            "lat": np.ascontiguousarray(latents[i]),
            "ctx": np.ascontiguousarray(context[i]),
            "wq": Wq, "wk": Wk, "wv": Wv, "wo": Wo, "ident": ident,
        }
        for i in range(8)
    ]
    res = run_bass_kernel_spmd(nc, in_maps, list(range(8)))
    return np.stack([res.results[i]["out"] for i in range(8)], axis=0)


# revision 16
# speedup vs baseline: 1.1969x; 1.1061x over previous
import sys

sys.path.insert(0, "/opt/trn_rl_repo")
import numpy as np
from contextlib import ExitStack

import concourse.bass as bass
import concourse.bacc as bacc
import concourse.tile as tile
from concourse import mybir
from concourse.bass_utils import run_bass_kernel_spmd

fp32 = mybir.dt.float32
fp32r = mybir.dt.float32r
ts = bass.ts
Exp = mybir.ActivationFunctionType.Exp
ADD = mybir.AluOpType.add
MULT = mybir.AluOpType.mult

# PerceiverAttention, GQA. Hardcoded problem shapes.
H = 2048            # hidden
NH = 32             # query heads
D = 64              # head dim
G = 8               # kv groups (rep=4, repeat_interleave: head h -> group h//4)
DKV = G * D         # 512
NLAT = 64
SEQ = 4096
KV = SEQ + NLAT     # 4160 = [context; latents]
HC = H // 128       # 16 h-chunks
SUP = 512           # kv rows per streamed super-block
NSUP = SEQ // SUP   # 8
ESC = 0.125         # 1/sqrt(head_dim)

_cache = {}


def _build():
    nc = bacc.Bacc("TRN2", target_bir_lowering=False, debug=False, num_devices=1)
    lat_d = nc.dram_tensor("lat", [NLAT, H], fp32, kind="ExternalInput").ap()
    ctx_d = nc.dram_tensor("ctx", [SEQ, H], fp32, kind="ExternalInput").ap()
    wq_d = nc.dram_tensor("wq", [H, H], fp32, kind="ExternalInput").ap()
    wk_d = nc.dram_tensor("wk", [H, DKV], fp32, kind="ExternalInput").ap()
    wv_d = nc.dram_tensor("wv", [H, DKV], fp32, kind="ExternalInput").ap()
    wo_d = nc.dram_tensor("wo", [H, H], fp32, kind="ExternalInput").ap()
    id_d = nc.dram_tensor("ident", [128, 128], fp32, kind="ExternalInput").ap()
    out_d = nc.dram_tensor("out", [NLAT, H], fp32, kind="ExternalOutput").ap()

    with tile.TileContext(nc) as tc, ExitStack() as ctx:
        sb1 = ctx.enter_context(tc.tile_pool(name="sb1", bufs=1))
        sb2 = ctx.enter_context(tc.tile_pool(name="sb2", bufs=2))
        ps = ctx.enter_context(tc.tile_pool(name="ps", bufs=2, space="PSUM"))

        # persistent SBUF
        wk_sb = sb1.tile([128, HC, DKV], fp32r)       # [h-part, hc, d_kv]
        wv_sb = sb1.tile([128, HC, DKV], fp32r)
        latT = sb1.tile([128, HC, NLAT], fp32r)       # lat^T
        qT = sb1.tile([128, 4, 256], fp32r)           # chunk c: groups 2c @p0, 2c+1 @p64
        o_acc = sb1.tile([128, G, 256], fp32)         # rows 0:64 attn-out^T, row 64 denom
        ctxT = sb1.tile([128, HC, SUP], fp32r)        # ctx^T current super
        kT = sb1.tile([128, 4, SUP], fp32r)           # K^T current super
        v1 = sb1.tile([128, 4, G, D + 1], fp32r)      # V' per kv-subchunk/group, col 64 ones
        kTt = sb1.tile([128, 4, NLAT], fp32r)         # K_tail^T (latent kv rows)
        v1t = sb1.tile([NLAT, G, D + 1], fp32r)
        id32 = sb1.tile([128, 128], fp32)
        id_r = sb1.tile([128, 128], fp32r)
        att = sb1.tile([128, HC, NLAT], fp32r)        # attn^T assembled
        den = sb1.tile([1, 1024], fp32)
        ones1 = sb1.tile([1, NLAT], fp32)
        onesc = sb1.tile([128, 1], fp32)

        def stg():
            return sb2.tile([128, H], fp32, tag="stg", name="stg")

        def ctr():
            return sb2.tile([128, H], fp32r, tag="ctr", name="ctr")

        def wst():
            return sb2.tile([128, 512], fp32, tag="wst", name="wst")

        def wr():
            return sb2.tile([128, 512], fp32r, tag="wr", name="wr")

        def qr():
            return sb2.tile([NLAT, 512], fp32r, tag="qr", name="qr")

        def et():
            return sb2.tile([128, 512], fp32r, tag="et", name="et")

        def outst():
            return sb2.tile([NLAT, 512], fp32, tag="outst", name="outst")

        def p(tag):
            return ps.tile([128, 512], fp32, tag=tag, name=tag)

        def ptr():
            return ps.tile([128, 4, 128], fp32r, tag="tr", name="tr")

        # ---- startup DMAs -------------------------------------------------
        # sync queue: ident, lat, then ctx supers (in super loop)
        # scalar queue: wk, wv, then wo halves (at tail)
        # gpsimd queue: wq
        nc.sync.dma_start(out=id32, in_=id_d)
        nc.vector.tensor_copy(out=id_r, in_=id32)
        lat_st = stg()
        nc.sync.dma_start(out=lat_st[0:NLAT, :], in_=lat_d)
        lat_r = ctr()
        nc.vector.tensor_copy(out=lat_r[0:NLAT, :], in_=lat_st[0:NLAT, :])

        nc.vector.memset(o_acc, 0.0)
        nc.vector.memset(ones1, 1.0)
        nc.vector.memset(onesc, 1.0)
        for kc in range(4):
            for g in range(G):
                nc.scalar.copy(out=v1[:, kc, g, 64:65], in_=onesc)
        for g in range(G):
            nc.scalar.copy(out=v1t[:, g, 64:65], in_=onesc[0:NLAT, :])

        # Wk/Wv resident as fp32r (scalar queue DMA, DVE cast)
        for wd, wsb in ((wk_d, wk_sb), (wv_d, wv_sb)):
            for hc in range(HC):
                w = wst()
                nc.scalar.dma_start(out=w, in_=wd[ts(hc, 128), :])
                nc.vector.tensor_copy(out=wsb[:, hc, :], in_=w)

        # lat^T: fp32r transposes, batched drains
        for hq in range(4):
            tp = ptr()
            for i in range(4):
                hc = 4 * hq + i
                nc.tensor.transpose(tp[:, i, 0:NLAT],
                                    lat_r[0:NLAT, ts(hc, 128)],
                                    id_r[0:NLAT, 0:NLAT])
            nc.vector.tensor_copy(out=latT[:, ts(hq, 4), :], in_=tp[:, :, 0:NLAT])

        def super_kv(s):
            # stream ctx super s: DMA (sync q) -> cast (DVE) -> transpose (PE)
            # -> ctxT (DVE drain); then K-proj / V-proj.
            for kc in range(4):
                st = stg()
                nc.sync.dma_start(out=st, in_=ctx_d[ts(4 * s + kc, 128), :])
                cr = ctr()
                nc.vector.tensor_copy(out=cr, in_=st)
                for hq in range(4):
                    tp = ptr()
                    for i in range(4):
                        hc = 4 * hq + i
                        nc.tensor.transpose(tp[:, i, :], cr[:, ts(hc, 128)], id_r)
                    nc.vector.tensor_copy(out=ctxT[:, ts(hq, 4), ts(kc, 128)],
                                          in_=tp)
            for dc in range(4):
                kp = p("kv")
                for hc in range(HC):
                    nc.tensor.matmul(kp, wk_sb[:, hc, ts(dc, 128)], ctxT[:, hc, :],
                                     start=(hc == 0), stop=(hc == HC - 1))
                nc.scalar.copy(out=kT[:, dc, :], in_=kp)
            for kc in range(4):
                vp = p("kv")
                for hc in range(HC):
                    nc.tensor.matmul(vp, ctxT[:, hc, ts(kc, 128)], wv_sb[:, hc, :],
                                     start=(hc == 0), stop=(hc == HC - 1))
                for g in range(G):
                    nc.vector.tensor_copy(out=v1[:, kc, g, 0:D],
                                          in_=vp[:, ts(g, D)])

        def attend_super(s):
            # group order: qT chunks 2,3 arrive first (gpsimd wq stream)
            for g in (4, 5, 6, 7, 0, 1, 2, 3):
                c, b0 = g // 2, 64 * (g % 2)
                op = p("o")
                for jj in range(2):
                    sp = p("s")
                    for j2 in range(2):
                        j = 2 * jj + j2
                        nc.tensor.matmul(sp[:, ts(j2, 256)],
                                         kT[b0:b0 + 64, c, ts(j, 128)],
                                         qT[b0:b0 + 64, c, :],
                                         start=True, stop=True)
                    e = et()
                    nc.scalar.activation(out=e, in_=sp, func=Exp, scale=ESC)
                    for j2 in range(2):
                        j = 2 * jj + j2
                        nc.tensor.matmul(op[0:D + 1, 0:256], v1[:, j, g, :],
                                         e[:, ts(j2, 256)],
                                         start=(j == 0), stop=(j == 3))
                nc.vector.tensor_tensor(out=o_acc[0:D + 1, g, :],
                                        in0=o_acc[0:D + 1, g, :],
                                        in1=op[0:D + 1, 0:256], op=ADD)

        def q_and_tails():
            # Q = lat @ Wq (wq split: qo 2,3 on gpsimd from t=0; qo 0,1 on
            # scalar after wk/wv), then Q^T, then latent-kv tail K^T / V'.
            for qo in (2, 3, 0, 1):
                wq_eng = nc.gpsimd if qo >= 2 else nc.scalar
                qp = p("kv")
                for hc in range(HC):
                    w = wst()
                    wq_eng.dma_start(out=w, in_=wq_d[ts(hc, 128), ts(qo, 512)])
                    wq_r = wr()
                    nc.scalar.copy(out=wq_r, in_=w)
                    nc.tensor.matmul(qp[0:NLAT, :], latT[:, hc, :], wq_r,
                                     start=(hc == 0), stop=(hc == HC - 1))
                q_r = qr()
                nc.scalar.copy(out=q_r, in_=qp[0:NLAT, :])
                # heads 8*qo .. 8*qo+7 live in this 512-col chunk; chunk c==qo
                for half in range(2):
                    b0 = NLAT * half
                    tp = ptr()
                    for i in range(4):
                        nc.tensor.transpose(tp[0:NLAT, i, 0:NLAT],
                                            q_r[:, ts(4 * half + i, D)],
                                            id_r[0:NLAT, 0:NLAT])
                    for i in range(4):
                        nc.scalar.copy(out=qT[b0:b0 + NLAT, qo, ts(i, D)],
                                       in_=tp[0:NLAT, i, 0:NLAT])
            # K_tail
            ktp = p("kv")
            for hc in range(HC):
                nc.tensor.matmul(ktp[0:NLAT, :], latT[:, hc, :], wk_sb[:, hc, :],
                                 start=(hc == 0), stop=(hc == HC - 1))
            kt_f = qr()
            nc.scalar.copy(out=kt_f, in_=ktp[0:NLAT, :])
            tp = ptr()
            for dc in range(4):
                nc.tensor.transpose(tp[:, dc, 0:NLAT], kt_f[:, ts(dc, 128)],
                                    id_r[0:NLAT, 0:NLAT])
            nc.vector.tensor_copy(out=kTt, in_=tp[:, :, 0:NLAT])
            # V_tail
            vtp = p("kv")
            for hc in range(HC):
                nc.tensor.matmul(vtp[0:NLAT, :], latT[:, hc, :], wv_sb[:, hc, :],
                                 start=(hc == 0), stop=(hc == HC - 1))
            for g in range(G):
                nc.scalar.copy(out=v1t[:, g, 0:D], in_=vtp[0:NLAT, ts(g, D)])

        for s in range(NSUP):
            super_kv(s)
            if s == 0:
                q_and_tails()
            attend_super(s)

        # latent kv tail attend (kv_rows = 64)
        for g in range(G):
            c, b0 = g // 2, 64 * (g % 2)
            sp = p("s")
            nc.tensor.matmul(sp[0:NLAT, 0:256], kTt[b0:b0 + 64, c, :],
                             qT[b0:b0 + 64, c, :], start=True, stop=True)
            e = et()
            nc.scalar.activation(out=e[0:NLAT, 0:256], in_=sp[0:NLAT, 0:256],
                                 func=Exp, scale=ESC)
            op = p("o")
            nc.tensor.matmul(op[0:D + 1, 0:256], v1t[:, g, :],
                             e[0:NLAT, 0:256], start=True, stop=True)
            nc.vector.tensor_tensor(out=o_acc[0:D + 1, g, :],
                                    in0=o_acc[0:D + 1, g, :],
                                    in1=op[0:D + 1, 0:256], op=ADD)

        # normalize: attn = exp / denom (row 64 of o_acc); batched reciprocal
        for half in range(2):
            for gg in range(4):
                nc.scalar.copy(out=den[:, ts(gg, 256)],
                               in_=o_acc[64:65, 4 * half + gg, :])
            nc.vector.reciprocal(out=den, in_=den)
            for gg in range(4):
                g = 4 * half + gg
                bp = p("s")
                nc.tensor.matmul(bp[0:NLAT, 0:256], ones1, den[:, ts(gg, 256)],
                                 start=True, stop=True)
                nc.vector.tensor_tensor(out=o_acc[0:D, g, :],
                                        in0=o_acc[0:D, g, :],
                                        in1=bp[0:NLAT, 0:256], op=MULT)

        # assemble attn^T [h*64+d, lat] chunks (fp32r for Wo matmul)
        for c in range(HC):
            h0, h1 = 2 * c, 2 * c + 1
            nc.scalar.copy(out=att[0:64, c, :],
                           in_=o_acc[0:D, h0 // 4, ts(h0 % 4, 64)])
            nc.scalar.copy(out=att[64:128, c, :],
                           in_=o_acc[0:D, h1 // 4, ts(h1 % 4, 64)])

        # out = attn_flat @ Wo; Wo streamed on all 3 queues, DVE-cast to fp32r
        wo_engs = (nc.sync, nc.scalar, nc.gpsimd, nc.sync, nc.scalar)
        for qo in range(4):
            op = p("kv")
            for hc in range(HC):
                w = wst()
                eng = wo_engs[(qo * HC + hc) % len(wo_engs)]
                eng.dma_start(out=w, in_=wo_d[ts(hc, 128), ts(qo, 512)])
                w_r = wr()
                nc.vector.tensor_copy(out=w_r, in_=w)
                nc.tensor.matmul(op[0:NLAT, :], att[:, hc, :], w_r,
                                 start=(hc == 0), stop=(hc == HC - 1))
            o_sb = outst()
            nc.scalar.copy(out=o_sb, in_=op[0:NLAT, :])
            nc.sync.dma_start(out=out_d[:, ts(qo, 512)], in_=o_sb)

    nc.compile()
    return nc


def kernel(latents, context, Wq, Wk, Wv, Wo):
    if "nc" not in _cache:
        _cache["nc"] = _build()
    nc = _cache["nc"]
    ident = np.eye(128, dtype=np.float32)
    in_maps = [
        {
            "lat": np.ascontiguousarray(latents[i]),
            "ctx": np.ascontiguousarray(context[i]),
            "wq": Wq, "wk": Wk, "wv": Wv, "wo": Wo, "ident": ident,
        }
        for i in range(8)
    ]
    res = run_bass_kernel_spmd(nc, in_maps, list(range(8)))
    return np.stack([res.results[i]["out"] for i in range(8)], axis=0)
